# revision 14
# baseline (speedup 1.0000x reference)
"""DND-LSTM cell (retrieval kNN + LSTM gates) on 8 Trainium2 NeuronCores.

Strategy (sharding_hint): shard keys/vals along dict_len (L=100000) across the
8 cores, 12500 each (padded to 12544 with dummy unit keys, excluded from the
softmax sums via ragged matmul slices). Each core streams its keysT/vals shard
from HBM once (memory-bound regime) and computes, flash-softmax style:

  num_partial[b, h]  = sum_l exp(cos(q_b, k_l) - 1) * vals[l, h]
  den_partial[b]     = sum_l exp(cos(q_b, k_l) - 1)

(cosine <= 1 exactly, so "-1" replaces the running row-max of a standard
streaming softmax; num/den ratios are unchanged.) The small LSTM-gate GEMMs are
sharded over the hidden dim (each core computes the 5 gate slices for its 32
hidden columns). The host gathers: sums the 8 num/den partials (the all-reduce)
and applies the final elementwise combine.

Device dataflow per 2048-key block (per core):
  DMA  keysT [128, 2, 2048] fp32 + vals [128, 16, 256] fp32   (one DMA each)
  POOL kt16 = bf16(keysT); vb[:, :, 0:256] = bf16(vals)       (idle engine;
       col 256 of vb = 1.0 so the same matmul accumulates the denominator)
  DVE  sq = kt16 * kt16 (bf16 2x mode)
  PE   normsq chunks [1, 512] = ones.T @ sq                   (bf16 matmul)
  ACT/DVE copy chunks -> SBUF [1, 2048]; POOL-DMA reshape -> [16, 128];
  PE   tiny transpose -> [128, 16] psum
  ACT  rsq = exp(-0.5 * ln(normsq))      (rsqrt; everything on ACT uses ONE
       table - natural_log_exp_and_others - so no 1.3us table reloads)
  PE   simsT[l, b] = kt16_tile.T @ qnT                        (bf16, N=256)
  ACT  ex = exp(simsT * rsq[l] - 1) -> bf16                   (fused scale+bias)
  PE   av[b, 0:258] += ex_bhalf.T @ vb_tile                   (persistent PSUM)

All heavy matmuls are bf16 (fp32 inputs rounded on device; the retrieval
branch contributes ~3e-3 of the output magnitude so bf16 rounding is far
below tolerance), the LSTM-gate matmuls stay float32r. Sigmoid/tanh are
computed as exp/ln compositions to stay on the single ACT table and avoid
custom DVE ucode. The only host arithmetic is the 8-way partial sum + final
elementwise LSTM combine (~0.002% of FLOPs).
"""

import ml_dtypes
import numpy as np

import concourse.bacc as bacc
import concourse.hw_specs as hw_specs
import concourse.masks as masks
import concourse.mybir as mybir
import concourse.tile as tile
from concourse import bass_utils

F32 = mybir.dt.float32
F32R = mybir.dt.float32r
BF16 = mybir.dt.bfloat16
AF = mybir.ActivationFunctionType

B = 256
D = 256
H = 256
NCORES = 8
HS = H // NCORES          # 32 hidden cols per core
GS = 5 * HS               # 160 gate cols per core
L = 100000
L_LOC = L // NCORES       # 12500 real keys per core
BLK = 2048                # keys per stream block
LPAD = ((L_LOC + 127) // 128) * 128  # 12544
NT_MAX = BLK // 128       # 16 l-tiles per full block

_TABLES_PATCHED = False


def _patch_act_tables():
    """Make every ACT function resolve to the one table that holds Ln, Exp,
    Square and Copy together (natural_log_exp_and_others). The default
    first-fit choice alternates between two tables, costing a 1.28us
    ACT_TABLE_LOAD per switch inside the hot loop. Table *indices* are
    preserved (ids index act_info.json), only membership is masked."""
    global _TABLES_PATCHED
    if _TABLES_PATCHED:
        return
    _TABLES_PATCHED = True
    orig = bacc.get_activation_tables

    def patched(arch):
        t = dict(orig(arch))
        keep = "natural_log_exp_and_others"
        if keep in t:
            t = {name: (funcs if name == keep else set())
                 for name, funcs in t.items()}
        return t

    bacc.get_activation_tables = patched


def _build(l_real=L_LOC, lpad=LPAD, blk=BLK):
    """Emit the per-core Bass program (identical on all 8 cores; all per-core
    variation is in the input data)."""
    _patch_act_tables()
    nt_max = blk // 128
    nblk_full = lpad // blk
    tail = lpad - nblk_full * blk
    blocks = [blk] * nblk_full + ([tail] if tail else [])

    nc = bacc.Bacc("TRN2", target_bir_lowering=False, debug=False,
                   num_devices=NCORES)

    keysT = nc.dram_tensor("keysT", [D, lpad], F32, kind="ExternalInput")
    vals = nc.dram_tensor("vals", [lpad, H], F32, kind="ExternalInput")
    x_t = nc.dram_tensor("x_t", [B, D], F32, kind="ExternalInput")
    xT_aug = nc.dram_tensor("xT_aug", [D + 2, B], F32R, kind="ExternalInput")
    hT = nc.dram_tensor("hT", [H, B], F32R, kind="ExternalInput")
    WiT = nc.dram_tensor("WiT", [D + 2, GS], F32R, kind="ExternalInput")
    WhT = nc.dram_tensor("WhT", [H, GS], F32R, kind="ExternalInput")
    c_sl = nc.dram_tensor("c_sl", [B, HS], F32, kind="ExternalInput")
    onesc = nc.dram_tensor("onesc", [128, 32], BF16, kind="ExternalInput")

    nd = nc.dram_tensor("nd", [B, H + 2], F32, kind="ExternalOutput")
    org = nc.dram_tensor("org", [B, 3 * HS], F32, kind="ExternalOutput")

    with tile.TileContext(nc) as tc:
        with (
            tc.tile_pool(name="const", bufs=1) as const,
            tc.tile_pool(name="sbA", bufs=2) as sbA,
            tc.tile_pool(name="psA", bufs=1, space="PSUM") as psA,
            tc.tile_pool(name="kpool", bufs=2) as kpool,
            tc.tile_pool(name="k16pool", bufs=2) as k16pool,
            tc.tile_pool(name="vlpool", bufs=2) as vlpool,
            tc.tile_pool(name="sqpool", bufs=2) as sqpool,
            tc.tile_pool(name="nqps", bufs=2, space="PSUM") as nqps,
            tc.tile_pool(name="rqpool", bufs=2) as rqpool,
            tc.tile_pool(name="smps", bufs=3, space="PSUM") as smps,
            tc.tile_pool(name="expool", bufs=6) as expool,
            tc.tile_pool(name="avps", bufs=1, space="PSUM") as avps,
            tc.tile_pool(name="vring", bufs=1) as vring,
        ):
            # --- constants ---
            ident = const.tile([128, 128], F32)
            masks.make_identity(nc, ident[:])
            ones32 = const.tile([128, 32], BF16)
            nc.gpsimd.dma_start(ones32[:], onesc.ap()[:])
            cm1 = const.tile([128, 1], F32)
            nc.vector.memset(cm1[:], -1.0)
            cm2 = const.tile([128, 1], F32)
            nc.vector.memset(cm2[:], -2.0)
            cm05 = const.tile([128, 1], F32)
            nc.vector.memset(cm05[:], -0.5)

            # persistent vals ring: [128, nt, 258]; col 256 = 1.0 (denominator)
            vbufs = []
            for i in range(3):
                vb = vring.tile([128, nt_max, H + 2], BF16, tag=f"vb{i}",
                                name=f"vb{i}")
                nc.gpsimd.dma_start(
                    vb[:, 0:nt_max, H:H + 2],
                    onesc.ap()[:, 0:2 * nt_max].rearrange(
                        "p (t o) -> p t o", o=2))
                vbufs.append(vb)

            # --- phase A: qn = x / ||x||, then qnT via PE transpose ---
            qnT = [const.tile([128, B], BF16, tag=f"qnT{dc}", name=f"qnT{dc}")
                   for dc in range(2)]
            for bh in range(2):
                xt = sbA.tile([128, D], F32, tag="xt")
                nc.gpsimd.dma_start(xt[:],
                                    x_t.ap()[bh * 128:(bh + 1) * 128, :])
                scr = sbA.tile([128, D], F32, tag="scr")
                nsq = sbA.tile([128, 1], F32, tag="nsq")
                nc.scalar.activation(scr[:], xt[:], AF.Square,
                                     accum_out=nsq[:])
                lnx = sbA.tile([128, 1], F32, tag="lnx")
                nc.scalar.activation(lnx[:], nsq[:], AF.Ln)
                rsx = sbA.tile([128, 1], F32, tag="rsx")
                nc.scalar.activation(rsx[:], lnx[:], AF.Exp, scale=cm05[:])
                qn = sbA.tile([128, D], F32, tag="qn")
                nc.vector.tensor_scalar_mul(qn[:], xt[:], rsx[:])
                for dc in range(2):
                    tp = psA.tile([128, 128], F32, tag="ps_scratch", name="tp")
                    nc.tensor.transpose(
                        tp[:], qn[:, dc * 128:(dc + 1) * 128], ident[:])
                    nc.vector.tensor_copy(
                        qnT[dc][:, bh * 128:(bh + 1) * 128], tp[:])

            # --- phase B: LSTM gate slices (this core's 32 hidden cols) ---
            xa = [sbA.tile([128, B], F32R, tag=f"xa{i}", name=f"xa{i}")
                  for i in range(2)]
            xa2 = sbA.tile([2, B], F32R, tag="xa2")
            ha = [sbA.tile([128, B], F32R, tag=f"ha{i}", name=f"ha{i}")
                  for i in range(2)]
            wi = [sbA.tile([128, GS], F32R, tag=f"wi{i}", name=f"wi{i}")
                  for i in range(2)]
            wi2 = sbA.tile([2, GS], F32R, tag="wi2")
            wh = [sbA.tile([128, GS], F32R, tag=f"wh{i}", name=f"wh{i}")
                  for i in range(2)]
            ctile = [sbA.tile([128, HS], F32, tag=f"ct{i}", name=f"ct{i}")
                     for i in range(2)]
            for i in range(2):
                nc.gpsimd.dma_start(xa[i][:],
                                    xT_aug.ap()[i * 128:(i + 1) * 128, :])
                nc.gpsimd.dma_start(ha[i][:],
                                    hT.ap()[i * 128:(i + 1) * 128, :])
                nc.gpsimd.dma_start(wi[i][:],
                                    WiT.ap()[i * 128:(i + 1) * 128, :])
                nc.gpsimd.dma_start(wh[i][:],
                                    WhT.ap()[i * 128:(i + 1) * 128, :])
                nc.gpsimd.dma_start(
                    ctile[i][:], c_sl.ap()[i * 128:(i + 1) * 128, :])
            nc.gpsimd.dma_start(xa2[:], xT_aug.ap()[256:258, :])
            nc.gpsimd.dma_start(wi2[:], WiT.ap()[256:258, :])

            for bh in range(2):
                bsl = slice(bh * 128, (bh + 1) * 128)
                pre = psA.tile([128, GS], F32, tag="ps_scratch", name="pre")
                nc.tensor.matmul(pre[:], xa[0][:, bsl], wi[0][:],
                                 start=True, stop=False)
                nc.tensor.matmul(pre[:], xa[1][:, bsl], wi[1][:],
                                 start=False, stop=False)
                nc.tensor.matmul(pre[:], xa2[:, bsl], wi2[:],
                                 start=False, stop=False)
                nc.tensor.matmul(pre[:], ha[0][:, bsl], wh[0][:],
                                 start=False, stop=False)
                nc.tensor.matmul(pre[:], ha[1][:, bsl], wh[1][:],
                                 start=False, stop=True)
                gates = sbA.tile([128, GS], F32, tag="gates")
                # sigmoid(x) = exp(-ln(1 + exp(-x))): stays on the Ln/Exp ACT
                # table and avoids custom DVE ucode (reciprocal) entirely
                e1 = sbA.tile([128, 128], F32, tag="e1")
                nc.scalar.activation(e1[:], pre[:, 0:128], AF.Exp, scale=cm1[:])
                nc.vector.tensor_scalar_add(e1[:], e1[:], 1.0)
                l1 = sbA.tile([128, 128], F32, tag="l1")
                nc.scalar.activation(l1[:], e1[:], AF.Ln)
                nc.scalar.activation(gates[:, 0:128], l1[:], AF.Exp,
                                     scale=cm1[:])
                # tanh(x) = 2 * sigmoid(2x) - 1
                e2 = sbA.tile([128, HS], F32, tag="e2")
                nc.scalar.activation(e2[:], pre[:, 128:160], AF.Exp,
                                     scale=cm2[:])
                nc.vector.tensor_scalar_add(e2[:], e2[:], 1.0)
                l2 = sbA.tile([128, HS], F32, tag="l2")
                nc.scalar.activation(l2[:], e2[:], AF.Ln)
                e3 = sbA.tile([128, HS], F32, tag="e3")
                nc.scalar.activation(e3[:], l2[:], AF.Exp, scale=cm1[:])
                nc.vector.tensor_scalar(
                    gates[:, 128:160], e3[:], 2.0, -1.0,
                    op0=mybir.AluOpType.mult, op1=mybir.AluOpType.add)
                # c_part = f*c + i*c~
                fc = sbA.tile([128, HS], F32, tag="fc")
                nc.vector.tensor_mul(fc[:], gates[:, 0:HS], ctile[bh][:])
                ic = sbA.tile([128, HS], F32, tag="ic")
                nc.vector.tensor_mul(ic[:], gates[:, HS:2 * HS],
                                     gates[:, 128:160])
                cp = sbA.tile([128, HS], F32, tag="cp")
                nc.vector.tensor_add(cp[:], fc[:], ic[:])
                nc.gpsimd.dma_start(org.ap()[bsl, 0:HS],
                                    gates[:, 2 * HS:3 * HS])      # o
                nc.gpsimd.dma_start(org.ap()[bsl, HS:2 * HS],
                                    gates[:, 3 * HS:4 * HS])      # r
                nc.gpsimd.dma_start(org.ap()[bsl, 2 * HS:3 * HS], cp[:])

            # --- phase C: stream the kNN retrieval ---
            av = [avps.tile([128, H + 2], F32, tag=f"av{bh}", name=f"av{bh}")
                  for bh in range(2)]
            total_tiles = lpad // 128
            tile_idx = 0
            for bi, bs in enumerate(blocks):
                off = bi * blk
                nt = bs // 128
                kt = kpool.tile([128, 2, bs], F32, tag="kt")
                nc.sync.dma_start(
                    kt[:],
                    keysT.ap()[:, off:off + bs].rearrange(
                        "(c p) l -> p c l", p=128))
                kt16 = k16pool.tile([128, 2, bs], BF16, tag="kt16")
                nc.gpsimd.tensor_copy(kt16[:], kt[:])
                vload = vlpool.tile([128, nt_max, H], F32, tag="vload")
                nc.sync.dma_start(
                    vload[:, 0:nt, :],
                    vals.ap()[off:off + bs, :].rearrange(
                        "(t p) h -> p t h", p=128))
                vb = vbufs[bi % 3]
                # split the cast between POOL and DVE to balance engine load
                half = (nt + 1) // 2
                nc.gpsimd.tensor_copy(vb[:, 0:half, 0:H],
                                      vload[:, 0:half, :])
                if nt > half:
                    nc.vector.tensor_copy(vb[:, half:nt, 0:H],
                                          vload[:, half:nt, :])
                sq = sqpool.tile([128, 2, bs], BF16, tag="sq")
                nc.vector.tensor_mul(sq[:], kt16[:], kt16[:])
                # normsq[l] -> rsq[p, t] (= 1/||k_l||, l = 128*t + p):
                # chunk sums [1, 512] -> SBUF [1, bs] -> reshape-DMA [nt, 128]
                # -> tiny PE transpose -> [128, nt] psum -> ACT rsqrt
                chunks = [(j0, min(512, bs - j0)) for j0 in range(0, bs, 512)]
                nqs = rqpool.tile([1, 2048], F32, tag="nqs")
                for j, (j0, cs) in enumerate(chunks):
                    nq = nqps.tile([1, 512], F32, tag="nq")
                    nc.tensor.matmul(nq[:, 0:cs], ones32[:, 0:1],
                                     sq[:, 0, j0:j0 + cs],
                                     start=True, stop=False)
                    nc.tensor.matmul(nq[:, 0:cs], ones32[:, 0:1],
                                     sq[:, 1, j0:j0 + cs],
                                     start=False, stop=True)
                    if j % 2 == 0:
                        nc.vector.tensor_copy(nqs[:, j0:j0 + cs], nq[:, 0:cs])
                    else:
                        nc.scalar.copy(nqs[:, j0:j0 + cs], nq[:, 0:cs])
                t4 = rqpool.tile([16, 128], F32, tag="t4")
                nc.gpsimd.dma_start(
                    t4[0:nt, :],
                    nqs[0:1, 0:bs].rearrange("o (r p) -> o r p", p=128))
                tpn = psA.tile([128, 16], F32, tag="ps_scratch", name="tpn")
                nc.tensor.transpose(tpn[:, 0:nt], t4[0:nt, :],
                                    ident[0:nt, 0:nt])
                rsq = rqpool.tile([128, nt_max], F32, tag="rsq")
                rln = rqpool.tile([128, nt_max], F32, tag="rln")
                nc.scalar.activation(rln[:, 0:nt], tpn[:, 0:nt], AF.Ln)
                nc.scalar.activation(rsq[:, 0:nt], rln[:, 0:nt], AF.Exp,
                                     scale=cm05[:])
                for t in range(nt):
                    l0 = off + t * 128
                    real = min(128, max(0, l_real - l0))
                    sm = smps.tile([128, B], F32, tag="sm")
                    nc.tensor.matmul(sm[:], kt16[:, 0, t * 128:(t + 1) * 128],
                                     qnT[0][:], start=True, stop=False)
                    nc.tensor.matmul(sm[:], kt16[:, 1, t * 128:(t + 1) * 128],
                                     qnT[1][:], start=False, stop=True)
                    ex = expool.tile([128, B], BF16, tag="ex")
                    nc.scalar.activation(ex[:], sm[:], AF.Exp,
                                         bias=cm1[:], scale=rsq[:, t:t + 1])
                    if real <= 0:
                        tile_idx += 1
                        continue
                    first = tile_idx == 0
                    last = tile_idx == total_tiles - 1
                    for bh in range(2):
                        nc.tensor.matmul(
                            av[bh][:],
                            ex[0:real, bh * 128:(bh + 1) * 128],
                            vb[0:real, t, :],
                            start=first, stop=last)
                    tile_idx += 1

            for bh in range(2):
                avs = sbA.tile([128, H + 2], F32, tag="avs")
                nc.vector.tensor_copy(avs[:], av[bh][:])
                nc.sync.dma_start(nd.ap()[bh * 128:(bh + 1) * 128, :],
                                  avs[:])

    nc.compile()
    return nc


_NC_CACHE = {}


def _get_nc():
    if "nc" not in _NC_CACHE:
        _NC_CACHE["nc"] = _build()
    return _NC_CACHE["nc"]


def _shard_inputs(x_t, h, c, W_i2h, b_i2h, W_h2h, b_h2h, keys, vals):
    f = np.float32
    x_t = np.ascontiguousarray(np.asarray(x_t, f))
    h = np.asarray(h, f)
    c = np.asarray(c, f)
    W_i2h = np.asarray(W_i2h, f)
    b_i2h = np.asarray(b_i2h, f)
    W_h2h = np.asarray(W_h2h, f)
    b_h2h = np.asarray(b_h2h, f)
    keys = np.asarray(keys, f)
    vals = np.asarray(vals, f)

    xT_aug = np.ascontiguousarray(
        np.concatenate([x_t.T, np.ones((2, B), f)], axis=0))
    hT = np.ascontiguousarray(h.T)
    WiT_full = W_i2h.T  # [D, G]
    WhT_full = W_h2h.T  # [H, G]

    in_maps = []
    for k in range(NCORES):
        sl = slice(k * L_LOC, (k + 1) * L_LOC)
        keysT = np.zeros((D, LPAD), f)
        keysT[:, :L_LOC] = keys[sl].T
        keysT[0, L_LOC:] = 1.0  # dummy unit keys (excluded from the sums)
        vpad = np.zeros((LPAD, H), f)
        vpad[:L_LOC] = vals[sl]
        gcols = np.concatenate(
            [np.arange(j * H + k * HS, j * H + (k + 1) * HS)
             for j in range(5)])
        WiT = np.concatenate(
            [WiT_full[:, gcols], b_i2h[gcols][None, :],
             b_h2h[gcols][None, :]], axis=0)
        in_maps.append({
            "onesc": np.ones((128, 32), ml_dtypes.bfloat16),
            "keysT": np.ascontiguousarray(keysT),
            "vals": np.ascontiguousarray(vpad),
            "x_t": x_t,
            "xT_aug": xT_aug,
            "hT": hT,
            "WiT": np.ascontiguousarray(WiT),
            "WhT": np.ascontiguousarray(WhT_full[:, gcols]),
            "c_sl": np.ascontiguousarray(c[:, k * HS:(k + 1) * HS]),
        })
    return in_maps


def kernel(x_t, h, c, W_i2h, b_i2h, W_h2h, b_h2h, keys, vals):
    nc = _get_nc()
    in_maps = _shard_inputs(x_t, h, c, W_i2h, b_i2h, W_h2h, b_h2h, keys, vals)
    res = bass_utils.run_bass_kernel_spmd(
        nc, in_maps, core_ids=list(range(NCORES)))

    num = np.zeros((B, H), np.float64)
    den = np.zeros((B,), np.float64)
    for k in range(NCORES):
        ndk = res.results[k]["nd"]
        num += ndk[:, :H]
        den += ndk[:, H]
    m = np.tanh(num / den[:, None]).astype(np.float32)

    h_t = np.empty((B, H), np.float32)
    c_t = np.empty((B, H), np.float32)
    for k in range(NCORES):
        orgk = res.results[k]["org"]
        o = orgk[:, 0:HS]
        r = orgk[:, HS:2 * HS]
        cp = orgk[:, 2 * HS:3 * HS]
        hs = slice(k * HS, (k + 1) * HS)
        ct = cp + r * m[:, hs]
        c_t[:, hs] = ct
        h_t[:, hs] = o * np.tanh(ct)
    return (h_t, c_t)


# revision 15
# speedup vs baseline: 1.2261x; 1.2261x over previous
"""DND-LSTM cell (retrieval kNN + LSTM gates) on 8 Trainium2 NeuronCores.

Strategy (sharding_hint): shard keys/vals along dict_len (L=100000) across the
8 cores, 12500 each (padded to 12544 with dummy unit keys, excluded from the
softmax sums via ragged matmul slices). Each core streams its keysT/vals shard
from HBM once (memory-bound regime) and computes, flash-softmax style:

  num_partial[b, h]  = sum_l exp(cos(q_b, k_l) - 1) * vals[l, h]
  den_partial[b]     = sum_l exp(cos(q_b, k_l) - 1)

(cosine <= 1 exactly, so "-1" replaces the running row-max of a standard
streaming softmax; num/den ratios are unchanged.) The small LSTM-gate GEMMs are
sharded over the hidden dim (each core computes the 5 gate slices for its 32
hidden columns). The host gathers: sums the 8 num/den partials (the all-reduce)
and applies the final elementwise combine.

Device dataflow per 2048-key block (per core):
  DMA  keysT [128, 2, 2048] fp32 + vals [128, 16, 256] fp32   (one DMA each)
  POOL kt16 = bf16(keysT); vb[:, :, 0:256] = bf16(vals)       (idle engine;
       col 256 of vb = 1.0 so the same matmul accumulates the denominator)
  DVE  sq = kt16 * kt16 (bf16 2x mode)
  PE   normsq chunks [1, 512] = ones.T @ sq                   (bf16 matmul)
  ACT/DVE copy chunks -> SBUF [1, 2048]; POOL-DMA reshape -> [16, 128];
  PE   tiny transpose -> [128, 16] psum
  ACT  rsq = exp(-0.5 * ln(normsq))      (rsqrt; everything on ACT uses ONE
       table - natural_log_exp_and_others - so no 1.3us table reloads)
  PE   simsT[l, b] = kt16_tile.T @ qnT                        (bf16, N=256)
  ACT  ex = exp(simsT * rsq[l] - 1) -> bf16                   (fused scale+bias)
  PE   av[b, 0:258] += ex_bhalf.T @ vb_tile                   (persistent PSUM)

All heavy matmuls are bf16 (fp32 inputs rounded on device; the retrieval
branch contributes ~3e-3 of the output magnitude so bf16 rounding is far
below tolerance), the LSTM-gate matmuls stay float32r. Sigmoid/tanh are
computed as exp/ln compositions to stay on the single ACT table and avoid
custom DVE ucode. The only host arithmetic is the 8-way partial sum + final
elementwise LSTM combine (~0.002% of FLOPs).
"""

import ml_dtypes
import numpy as np

import concourse.bacc as bacc
import concourse.hw_specs as hw_specs
import concourse.masks as masks
import concourse.mybir as mybir
import concourse.tile as tile
from concourse import bass_utils

F32 = mybir.dt.float32
F32R = mybir.dt.float32r
BF16 = mybir.dt.bfloat16
AF = mybir.ActivationFunctionType

B = 256
D = 256
H = 256
NCORES = 8
HS = H // NCORES          # 32 hidden cols per core
GS = 5 * HS               # 160 gate cols per core
L = 100000
L_LOC = L // NCORES       # 12500 real keys per core
BLK = 2048                # keys per stream block
LPAD = ((L_LOC + 127) // 128) * 128  # 12544
NT_MAX = BLK // 128       # 16 l-tiles per full block

_TABLES_PATCHED = False


def _patch_act_tables():
    """Make every ACT function resolve to the one table that holds Ln, Exp,
    Square and Copy together (natural_log_exp_and_others). The default
    first-fit choice alternates between two tables, costing a 1.28us
    ACT_TABLE_LOAD per switch inside the hot loop. Table *indices* are
    preserved (ids index act_info.json), only membership is masked."""
    global _TABLES_PATCHED
    if _TABLES_PATCHED:
        return
    _TABLES_PATCHED = True
    orig = bacc.get_activation_tables

    def patched(arch):
        t = dict(orig(arch))
        keep = "natural_log_exp_and_others"
        if keep in t:
            t = {name: (funcs if name == keep else set())
                 for name, funcs in t.items()}
        return t

    bacc.get_activation_tables = patched


def _build(l_real=L_LOC, lpad=LPAD, blk=BLK):
    """Emit the per-core Bass program (identical on all 8 cores; all per-core
    variation is in the input data)."""
    _patch_act_tables()
    nt_max = blk // 128
    nblk_full = lpad // blk
    tail = lpad - nblk_full * blk
    blocks = [blk] * nblk_full + ([tail] if tail else [])

    nc = bacc.Bacc("TRN2", target_bir_lowering=False, debug=False,
                   num_devices=NCORES)

    keysT = nc.dram_tensor("keysT", [D, lpad], F32, kind="ExternalInput")
    nblk = len(blocks)
    vals2 = nc.dram_tensor("vals2", [nblk, 128, nt_max * (H + 2)], F32R,
                           kind="ExternalInput")
    x_t = nc.dram_tensor("x_t", [B, D], F32, kind="ExternalInput")
    xT_aug = nc.dram_tensor("xT_aug", [D + 2, B], F32R, kind="ExternalInput")
    hT = nc.dram_tensor("hT", [H, B], F32R, kind="ExternalInput")
    WiT = nc.dram_tensor("WiT", [D + 2, GS], F32R, kind="ExternalInput")
    WhT = nc.dram_tensor("WhT", [H, GS], F32R, kind="ExternalInput")
    c_sl = nc.dram_tensor("c_sl", [B, HS], F32, kind="ExternalInput")
    onesc = nc.dram_tensor("onesc", [128, 32], BF16, kind="ExternalInput")

    nd = nc.dram_tensor("nd", [B, H + 2], F32, kind="ExternalOutput")
    org = nc.dram_tensor("org", [B, 3 * HS], F32, kind="ExternalOutput")

    with tile.TileContext(nc) as tc:
        with (
            tc.tile_pool(name="const", bufs=1) as const,
            tc.tile_pool(name="sbA", bufs=2) as sbA,
            tc.tile_pool(name="psA", bufs=1, space="PSUM") as psA,
            tc.tile_pool(name="kpool", bufs=2) as kpool,
            tc.tile_pool(name="k16pool", bufs=2) as k16pool,
            tc.tile_pool(name="vbpool", bufs=3) as vbpool,
            tc.tile_pool(name="sqpool", bufs=2) as sqpool,
            tc.tile_pool(name="nqps", bufs=2, space="PSUM") as nqps,
            tc.tile_pool(name="rqpool", bufs=2) as rqpool,
            tc.tile_pool(name="smps", bufs=3, space="PSUM") as smps,
            tc.tile_pool(name="expool", bufs=6) as expool,
            tc.tile_pool(name="avps", bufs=1, space="PSUM") as avps,
        ):
            # --- constants ---
            ident = const.tile([128, 128], F32)
            masks.make_identity(nc, ident[:])
            ones32 = const.tile([128, 32], BF16)
            nc.gpsimd.dma_start(ones32[:], onesc.ap()[:])
            cm1 = const.tile([128, 1], F32)
            nc.vector.memset(cm1[:], -1.0)
            cm2 = const.tile([128, 1], F32)
            nc.vector.memset(cm2[:], -2.0)
            cm05 = const.tile([128, 1], F32)
            nc.vector.memset(cm05[:], -0.5)

            # --- phase A: qn = x / ||x||, then qnT via PE transpose ---
            qnT = [const.tile([128, B], BF16, tag=f"qnT{dc}", name=f"qnT{dc}")
                   for dc in range(2)]
            for bh in range(2):
                xt = sbA.tile([128, D], F32, tag="xt")
                nc.gpsimd.dma_start(xt[:],
                                    x_t.ap()[bh * 128:(bh + 1) * 128, :])
                scr = sbA.tile([128, D], F32, tag="scr")
                nsq = sbA.tile([128, 1], F32, tag="nsq")
                nc.scalar.activation(scr[:], xt[:], AF.Square,
                                     accum_out=nsq[:])
                lnx = sbA.tile([128, 1], F32, tag="lnx")
                nc.scalar.activation(lnx[:], nsq[:], AF.Ln)
                rsx = sbA.tile([128, 1], F32, tag="rsx")
                nc.scalar.activation(rsx[:], lnx[:], AF.Exp, scale=cm05[:])
                qn = sbA.tile([128, D], F32, tag="qn")
                nc.vector.tensor_scalar_mul(qn[:], xt[:], rsx[:])
                for dc in range(2):
                    tp = psA.tile([128, 128], F32, tag="ps_scratch", name="tp")
                    nc.tensor.transpose(
                        tp[:], qn[:, dc * 128:(dc + 1) * 128], ident[:])
                    nc.vector.tensor_copy(
                        qnT[dc][:, bh * 128:(bh + 1) * 128], tp[:])

            # --- phase B: LSTM gate slices (this core's 32 hidden cols) ---
            xa = [sbA.tile([128, B], F32R, tag=f"xa{i}", name=f"xa{i}")
                  for i in range(2)]
            xa2 = sbA.tile([2, B], F32R, tag="xa2")
            ha = [sbA.tile([128, B], F32R, tag=f"ha{i}", name=f"ha{i}")
                  for i in range(2)]
            wi = [sbA.tile([128, GS], F32R, tag=f"wi{i}", name=f"wi{i}")
                  for i in range(2)]
            wi2 = sbA.tile([2, GS], F32R, tag="wi2")
            wh = [sbA.tile([128, GS], F32R, tag=f"wh{i}", name=f"wh{i}")
                  for i in range(2)]
            ctile = [sbA.tile([128, HS], F32, tag=f"ct{i}", name=f"ct{i}")
                     for i in range(2)]
            for i in range(2):
                nc.gpsimd.dma_start(xa[i][:],
                                    xT_aug.ap()[i * 128:(i + 1) * 128, :])
                nc.gpsimd.dma_start(ha[i][:],
                                    hT.ap()[i * 128:(i + 1) * 128, :])
                nc.gpsimd.dma_start(wi[i][:],
                                    WiT.ap()[i * 128:(i + 1) * 128, :])
                nc.gpsimd.dma_start(wh[i][:],
                                    WhT.ap()[i * 128:(i + 1) * 128, :])
                nc.gpsimd.dma_start(
                    ctile[i][:], c_sl.ap()[i * 128:(i + 1) * 128, :])
            nc.gpsimd.dma_start(xa2[:], xT_aug.ap()[256:258, :])
            nc.gpsimd.dma_start(wi2[:], WiT.ap()[256:258, :])

            for bh in range(2):
                bsl = slice(bh * 128, (bh + 1) * 128)
                pre = psA.tile([128, GS], F32, tag="ps_scratch", name="pre")
                nc.tensor.matmul(pre[:], xa[0][:, bsl], wi[0][:],
                                 start=True, stop=False)
                nc.tensor.matmul(pre[:], xa[1][:, bsl], wi[1][:],
                                 start=False, stop=False)
                nc.tensor.matmul(pre[:], xa2[:, bsl], wi2[:],
                                 start=False, stop=False)
                nc.tensor.matmul(pre[:], ha[0][:, bsl], wh[0][:],
                                 start=False, stop=False)
                nc.tensor.matmul(pre[:], ha[1][:, bsl], wh[1][:],
                                 start=False, stop=True)
                gates = sbA.tile([128, GS], F32, tag="gates")
                # sigmoid(x) = exp(-ln(1 + exp(-x))): stays on the Ln/Exp ACT
                # table and avoids custom DVE ucode (reciprocal) entirely
                e1 = sbA.tile([128, 128], F32, tag="e1")
                nc.scalar.activation(e1[:], pre[:, 0:128], AF.Exp, scale=cm1[:])
                nc.vector.tensor_scalar_add(e1[:], e1[:], 1.0)
                l1 = sbA.tile([128, 128], F32, tag="l1")
                nc.scalar.activation(l1[:], e1[:], AF.Ln)
                nc.scalar.activation(gates[:, 0:128], l1[:], AF.Exp,
                                     scale=cm1[:])
                # tanh(x) = 2 * sigmoid(2x) - 1
                e2 = sbA.tile([128, HS], F32, tag="e2")
                nc.scalar.activation(e2[:], pre[:, 128:160], AF.Exp,
                                     scale=cm2[:])
                nc.vector.tensor_scalar_add(e2[:], e2[:], 1.0)
                l2 = sbA.tile([128, HS], F32, tag="l2")
                nc.scalar.activation(l2[:], e2[:], AF.Ln)
                e3 = sbA.tile([128, HS], F32, tag="e3")
                nc.scalar.activation(e3[:], l2[:], AF.Exp, scale=cm1[:])
                nc.vector.tensor_scalar(
                    gates[:, 128:160], e3[:], 2.0, -1.0,
                    op0=mybir.AluOpType.mult, op1=mybir.AluOpType.add)
                # c_part = f*c + i*c~
                fc = sbA.tile([128, HS], F32, tag="fc")
                nc.vector.tensor_mul(fc[:], gates[:, 0:HS], ctile[bh][:])
                ic = sbA.tile([128, HS], F32, tag="ic")
                nc.vector.tensor_mul(ic[:], gates[:, HS:2 * HS],
                                     gates[:, 128:160])
                cp = sbA.tile([128, HS], F32, tag="cp")
                nc.vector.tensor_add(cp[:], fc[:], ic[:])
                nc.gpsimd.dma_start(org.ap()[bsl, 0:HS],
                                    gates[:, 2 * HS:3 * HS])      # o
                nc.gpsimd.dma_start(org.ap()[bsl, HS:2 * HS],
                                    gates[:, 3 * HS:4 * HS])      # r
                nc.gpsimd.dma_start(org.ap()[bsl, 2 * HS:3 * HS], cp[:])

            # --- phase C: stream the kNN retrieval ---
            av = [avps.tile([128, H + 2], F32, tag=f"av{bh}", name=f"av{bh}")
                  for bh in range(2)]
            total_tiles = lpad // 128
            tile_idx = 0
            for bi, bs in enumerate(blocks):
                off = bi * blk
                nt = bs // 128
                kt = kpool.tile([128, 2, bs], F32, tag="kt")
                nc.sync.dma_start(
                    kt[:],
                    keysT.ap()[:, off:off + bs].rearrange(
                        "(c p) l -> p c l", p=128))
                kt16 = k16pool.tile([128, 2, bs], BF16, tag="kt16")
                nc.vector.tensor_copy(kt16[:], kt[:])
                vb = vbpool.tile([128, nt_max * (H + 2)], F32R, tag="vb")
                nc.sync.dma_start(vb[:, 0:nt * (H + 2)],
                                  vals2.ap()[bi, :, 0:nt * (H + 2)])
                sq = sqpool.tile([128, 2, bs], BF16, tag="sq")
                nc.vector.tensor_mul(sq[:], kt16[:], kt16[:])
                # normsq[l] -> rsq[p, t] (= 1/||k_l||, l = 128*t + p):
                # chunk sums [1, 512] -> SBUF [1, bs] -> reshape-DMA [nt, 128]
                # -> tiny PE transpose -> [128, nt] psum -> ACT rsqrt
                chunks = [(j0, min(512, bs - j0)) for j0 in range(0, bs, 512)]
                nqs = rqpool.tile([1, 2048], F32, tag="nqs")
                for j, (j0, cs) in enumerate(chunks):
                    nq = nqps.tile([1, 512], F32, tag="nq")
                    nc.tensor.matmul(nq[:, 0:cs], ones32[:, 0:1],
                                     sq[:, 0, j0:j0 + cs],
                                     start=True, stop=False)
                    nc.tensor.matmul(nq[:, 0:cs], ones32[:, 0:1],
                                     sq[:, 1, j0:j0 + cs],
                                     start=False, stop=True)
                    if j % 2 == 0:
                        nc.vector.tensor_copy(nqs[:, j0:j0 + cs], nq[:, 0:cs])
                    else:
                        nc.scalar.copy(nqs[:, j0:j0 + cs], nq[:, 0:cs])
                t4 = rqpool.tile([16, 128], F32, tag="t4")
                nc.gpsimd.dma_start(
                    t4[0:nt, :],
                    nqs[0:1, 0:bs].rearrange("o (r p) -> o r p", p=128))
                tpn = psA.tile([128, 16], F32, tag="ps_scratch", name="tpn")
                nc.tensor.transpose(tpn[:, 0:nt], t4[0:nt, :],
                                    ident[0:nt, 0:nt])
                rsq = rqpool.tile([128, nt_max], F32, tag="rsq")
                rln = rqpool.tile([128, nt_max], F32, tag="rln")
                nc.scalar.activation(rln[:, 0:nt], tpn[:, 0:nt], AF.Ln)
                nc.scalar.activation(rsq[:, 0:nt], rln[:, 0:nt], AF.Exp,
                                     scale=cm05[:])
                for t in range(nt):
                    l0 = off + t * 128
                    real = min(128, max(0, l_real - l0))
                    sm = smps.tile([128, B], F32, tag="sm")
                    nc.tensor.matmul(sm[:], kt16[:, 0, t * 128:(t + 1) * 128],
                                     qnT[0][:], start=True, stop=False)
                    nc.tensor.matmul(sm[:], kt16[:, 1, t * 128:(t + 1) * 128],
                                     qnT[1][:], start=False, stop=True)
                    ex = expool.tile([128, B], F32R, tag="ex")
                    nc.scalar.activation(ex[:], sm[:], AF.Exp,
                                         bias=cm1[:], scale=rsq[:, t:t + 1])
                    if real <= 0:
                        tile_idx += 1
                        continue
                    first = tile_idx == 0
                    last = tile_idx == total_tiles - 1
                    for bh in range(2):
                        nc.tensor.matmul(
                            av[bh][:],
                            ex[0:real, bh * 128:(bh + 1) * 128],
                            vb[0:real, t * (H + 2):(t + 1) * (H + 2)],
                            start=first, stop=last)
                    tile_idx += 1

            for bh in range(2):
                avs = sbA.tile([128, H + 2], F32, tag="avs")
                nc.vector.tensor_copy(avs[:], av[bh][:])
                nc.sync.dma_start(nd.ap()[bh * 128:(bh + 1) * 128, :],
                                  avs[:])

    nc.compile()
    return nc


_NC_CACHE = {}


def _get_nc():
    if "nc" not in _NC_CACHE:
        _NC_CACHE["nc"] = _build()
    return _NC_CACHE["nc"]


def _shard_inputs(x_t, h, c, W_i2h, b_i2h, W_h2h, b_h2h, keys, vals):
    f = np.float32
    x_t = np.ascontiguousarray(np.asarray(x_t, f))
    h = np.asarray(h, f)
    c = np.asarray(c, f)
    W_i2h = np.asarray(W_i2h, f)
    b_i2h = np.asarray(b_i2h, f)
    W_h2h = np.asarray(W_h2h, f)
    b_h2h = np.asarray(b_h2h, f)
    keys = np.asarray(keys, f)
    vals = np.asarray(vals, f)

    xT_aug = np.ascontiguousarray(
        np.concatenate([x_t.T, np.ones((2, B), f)], axis=0))
    hT = np.ascontiguousarray(h.T)
    WiT_full = W_i2h.T  # [D, G]
    WhT_full = W_h2h.T  # [H, G]

    in_maps = []
    for k in range(NCORES):
        sl = slice(k * L_LOC, (k + 1) * L_LOC)
        keysT = np.zeros((D, LPAD), f)
        keysT[:, :L_LOC] = keys[sl].T
        keysT[0, L_LOC:] = 1.0  # dummy unit keys (excluded from the sums)
        vpad = np.zeros((LPAD, H + 2), f)
        vpad[:L_LOC, :H] = vals[sl]
        vpad[:L_LOC, H] = 1.0  # denominator column (excluded rows stay 0)
        nblk = (LPAD + BLK - 1) // BLK
        v2 = np.zeros((nblk, 128, NT_MAX * (H + 2)), f)
        for bi in range(nblk):
            bs = min(BLK, LPAD - bi * BLK)
            nt = bs // 128
            blkv = vpad[bi * BLK:bi * BLK + bs]          # [bs, 258]
            v2[bi, :, :nt * (H + 2)] = blkv.reshape(
                nt, 128, H + 2).transpose(1, 0, 2).reshape(128, nt * (H + 2))
        gcols = np.concatenate(
            [np.arange(j * H + k * HS, j * H + (k + 1) * HS)
             for j in range(5)])
        WiT = np.concatenate(
            [WiT_full[:, gcols], b_i2h[gcols][None, :],
             b_h2h[gcols][None, :]], axis=0)
        in_maps.append({
            "onesc": np.ones((128, 32), ml_dtypes.bfloat16),
            "keysT": np.ascontiguousarray(keysT),
            "vals2": v2,
            "x_t": x_t,
            "xT_aug": xT_aug,
            "hT": hT,
            "WiT": np.ascontiguousarray(WiT),
            "WhT": np.ascontiguousarray(WhT_full[:, gcols]),
            "c_sl": np.ascontiguousarray(c[:, k * HS:(k + 1) * HS]),
        })
    return in_maps


def kernel(x_t, h, c, W_i2h, b_i2h, W_h2h, b_h2h, keys, vals):
    nc = _get_nc()
    in_maps = _shard_inputs(x_t, h, c, W_i2h, b_i2h, W_h2h, b_h2h, keys, vals)
    res = bass_utils.run_bass_kernel_spmd(
        nc, in_maps, core_ids=list(range(NCORES)))

    num = np.zeros((B, H), np.float64)
    den = np.zeros((B,), np.float64)
    for k in range(NCORES):
        ndk = res.results[k]["nd"]
        num += ndk[:, :H]
        den += ndk[:, H]
    m = np.tanh(num / den[:, None]).astype(np.float32)

    h_t = np.empty((B, H), np.float32)
    c_t = np.empty((B, H), np.float32)
    for k in range(NCORES):
        orgk = res.results[k]["org"]
        o = orgk[:, 0:HS]
        r = orgk[:, HS:2 * HS]
        cp = orgk[:, 2 * HS:3 * HS]
        hs = slice(k * HS, (k + 1) * HS)
        ct = cp + r * m[:, hs]
        c_t[:, hs] = ct
        h_t[:, hs] = o * np.tanh(ct)
    return (h_t, c_t)


# revision 17
# speedup vs baseline: 1.3284x; 1.0835x over previous
"""DND-LSTM cell (retrieval kNN + LSTM gates) on 8 Trainium2 NeuronCores.

Strategy (sharding_hint): shard keys/vals along dict_len (L=100000) across the
8 cores, 12500 each (padded to 12544 with dummy unit keys, excluded from the
softmax sums via ragged matmul slices). Each core streams its keysT/vals shard
from HBM once (memory-bound regime) and computes, flash-softmax style:

  num_partial[b, h]  = sum_l exp(cos(q_b, k_l) - 1) * vals[l, h]
  den_partial[b]     = sum_l exp(cos(q_b, k_l) - 1)

(cosine <= 1 exactly, so "-1" replaces the running row-max of a standard
streaming softmax; num/den ratios are unchanged.) The small LSTM-gate GEMMs are
sharded over the hidden dim (each core computes the 5 gate slices for its 32
hidden columns). The host gathers: sums the 8 num/den partials (the all-reduce)
and applies the final elementwise combine.

Device dataflow per 2048-key block (per core):
  DMA  keysT [128, 2, 2048] fp32 + vals [128, 16, 256] fp32   (one DMA each)
  POOL kt16 = bf16(keysT); vb[:, :, 0:256] = bf16(vals)       (idle engine;
       col 256 of vb = 1.0 so the same matmul accumulates the denominator)
  DVE  sq = kt16 * kt16 (bf16 2x mode)
  PE   normsq chunks [1, 512] = ones.T @ sq                   (bf16 matmul)
  ACT/DVE copy chunks -> SBUF [1, 2048]; POOL-DMA reshape -> [16, 128];
  PE   tiny transpose -> [128, 16] psum
  ACT  rsq = exp(-0.5 * ln(normsq))      (rsqrt; everything on ACT uses ONE
       table - natural_log_exp_and_others - so no 1.3us table reloads)
  PE   simsT[l, b] = kt16_tile.T @ qnT                        (bf16, N=256)
  ACT  ex = exp(simsT * rsq[l] - 1) -> bf16                   (fused scale+bias)
  PE   av[b, 0:258] += ex_bhalf.T @ vb_tile                   (persistent PSUM)

All heavy matmuls are bf16 (fp32 inputs rounded on device; the retrieval
branch contributes ~3e-3 of the output magnitude so bf16 rounding is far
below tolerance), the LSTM-gate matmuls stay float32r. Sigmoid/tanh are
computed as exp/ln compositions to stay on the single ACT table and avoid
custom DVE ucode. The only host arithmetic is the 8-way partial sum + final
elementwise LSTM combine (~0.002% of FLOPs).
"""

import ml_dtypes
import numpy as np

import concourse.bacc as bacc
import concourse.hw_specs as hw_specs
import concourse.masks as masks
import concourse.mybir as mybir
import concourse.tile as tile
from concourse import bass_utils

F32 = mybir.dt.float32
F32R = mybir.dt.float32r
BF16 = mybir.dt.bfloat16
AF = mybir.ActivationFunctionType

B = 256
D = 256
H = 256
NCORES = 8
HS = H // NCORES          # 32 hidden cols per core
GS = 5 * HS               # 160 gate cols per core
L = 100000
L_LOC = L // NCORES       # 12500 real keys per core
BLK = 2048                # keys per stream block
LPAD = ((L_LOC + 127) // 128) * 128  # 12544
NT_MAX = BLK // 128       # 16 l-tiles per full block

_TABLES_PATCHED = False


def _patch_act_tables():
    """Make every ACT function resolve to the one table that holds Ln, Exp,
    Square and Copy together (natural_log_exp_and_others). The default
    first-fit choice alternates between two tables, costing a 1.28us
    ACT_TABLE_LOAD per switch inside the hot loop. Table *indices* are
    preserved (ids index act_info.json), only membership is masked."""
    global _TABLES_PATCHED
    if _TABLES_PATCHED:
        return
    _TABLES_PATCHED = True
    orig = bacc.get_activation_tables

    def patched(arch):
        t = dict(orig(arch))
        keep = "natural_log_exp_and_others"
        if keep in t:
            t = {name: (funcs if name == keep else set())
                 for name, funcs in t.items()}
        return t

    bacc.get_activation_tables = patched


def _build(l_real=L_LOC, lpad=LPAD, blk=BLK):
    """Emit the per-core Bass program (identical on all 8 cores; all per-core
    variation is in the input data)."""
    _patch_act_tables()
    nt_max = blk // 128
    nblk_full = lpad // blk
    tail = lpad - nblk_full * blk
    blocks = [blk] * nblk_full + ([tail] if tail else [])

    nc = bacc.Bacc("TRN2", target_bir_lowering=False, debug=False,
                   num_devices=NCORES)

    keysT = nc.dram_tensor("keysT", [D, lpad], F32, kind="ExternalInput")
    nblk = len(blocks)
    vals2 = nc.dram_tensor("vals2", [nblk, 128, nt_max * (H + 2)], F32,
                           kind="ExternalInput")
    x_t = nc.dram_tensor("x_t", [B, D], F32, kind="ExternalInput")
    xT_aug = nc.dram_tensor("xT_aug", [D + 2, B], F32R, kind="ExternalInput")
    hT = nc.dram_tensor("hT", [H, B], F32R, kind="ExternalInput")
    WiT = nc.dram_tensor("WiT", [D + 2, GS], F32R, kind="ExternalInput")
    WhT = nc.dram_tensor("WhT", [H, GS], F32R, kind="ExternalInput")
    c_sl = nc.dram_tensor("c_sl", [B, HS], F32, kind="ExternalInput")
    onesc = nc.dram_tensor("onesc", [128, 32], BF16, kind="ExternalInput")

    nd = nc.dram_tensor("nd", [B, H + 2], F32, kind="ExternalOutput")
    org = nc.dram_tensor("org", [B, 3 * HS], F32, kind="ExternalOutput")

    with tile.TileContext(nc) as tc:
        with (
            tc.tile_pool(name="const", bufs=1) as const,
            tc.tile_pool(name="sbA", bufs=2) as sbA,
            tc.tile_pool(name="psA", bufs=1, space="PSUM") as psA,
            tc.tile_pool(name="kpool", bufs=2) as kpool,
            tc.tile_pool(name="k16pool", bufs=2) as k16pool,
            tc.tile_pool(name="vbpool", bufs=2) as vbpool,
            tc.tile_pool(name="v16pool", bufs=3) as v16pool,
            tc.tile_pool(name="sqpool", bufs=2) as sqpool,
            tc.tile_pool(name="nqps", bufs=1, space="PSUM") as nqps,
            tc.tile_pool(name="rqpool", bufs=2) as rqpool,
            tc.tile_pool(name="smps", bufs=4, space="PSUM") as smps,
            tc.tile_pool(name="expool", bufs=6) as expool,
            tc.tile_pool(name="avps", bufs=1, space="PSUM") as avps,
        ):
            # --- constants ---
            ident = const.tile([128, 128], F32)
            masks.make_identity(nc, ident[:])
            ones32 = const.tile([128, 32], BF16)
            nc.gpsimd.dma_start(ones32[:], onesc.ap()[:])
            cm1 = const.tile([128, 1], F32)
            nc.vector.memset(cm1[:], -1.0)
            cm2 = const.tile([128, 1], F32)
            nc.vector.memset(cm2[:], -2.0)
            cm05 = const.tile([128, 1], F32)
            nc.vector.memset(cm05[:], -0.5)

            # --- phase A: qn = x / ||x||, then qnT via PE transpose ---
            qnT = [const.tile([128, B], BF16, tag=f"qnT{dc}", name=f"qnT{dc}")
                   for dc in range(2)]
            for bh in range(2):
                xt = sbA.tile([128, D], F32, tag="xt")
                nc.gpsimd.dma_start(xt[:],
                                    x_t.ap()[bh * 128:(bh + 1) * 128, :])
                scr = sbA.tile([128, D], F32, tag="scr")
                nsq = sbA.tile([128, 1], F32, tag="nsq")
                nc.scalar.activation(scr[:], xt[:], AF.Square,
                                     accum_out=nsq[:])
                lnx = sbA.tile([128, 1], F32, tag="lnx")
                nc.scalar.activation(lnx[:], nsq[:], AF.Ln)
                rsx = sbA.tile([128, 1], F32, tag="rsx")
                nc.scalar.activation(rsx[:], lnx[:], AF.Exp, scale=cm05[:])
                qn = sbA.tile([128, D], F32, tag="qn")
                nc.vector.tensor_scalar_mul(qn[:], xt[:], rsx[:])
                for dc in range(2):
                    tp = psA.tile([128, 128], F32, tag="ps_scratch", name="tp")
                    nc.tensor.transpose(
                        tp[:], qn[:, dc * 128:(dc + 1) * 128], ident[:])
                    nc.vector.tensor_copy(
                        qnT[dc][:, bh * 128:(bh + 1) * 128], tp[:])

            # --- phase B: LSTM gate slices (this core's 32 hidden cols) ---
            xa = [sbA.tile([128, B], F32R, tag=f"xa{i}", name=f"xa{i}")
                  for i in range(2)]
            xa2 = sbA.tile([2, B], F32R, tag="xa2")
            ha = [sbA.tile([128, B], F32R, tag=f"ha{i}", name=f"ha{i}")
                  for i in range(2)]
            wi = [sbA.tile([128, GS], F32R, tag=f"wi{i}", name=f"wi{i}")
                  for i in range(2)]
            wi2 = sbA.tile([2, GS], F32R, tag="wi2")
            wh = [sbA.tile([128, GS], F32R, tag=f"wh{i}", name=f"wh{i}")
                  for i in range(2)]
            ctile = [sbA.tile([128, HS], F32, tag=f"ct{i}", name=f"ct{i}")
                     for i in range(2)]
            for i in range(2):
                nc.gpsimd.dma_start(xa[i][:],
                                    xT_aug.ap()[i * 128:(i + 1) * 128, :])
                nc.gpsimd.dma_start(ha[i][:],
                                    hT.ap()[i * 128:(i + 1) * 128, :])
                nc.gpsimd.dma_start(wi[i][:],
                                    WiT.ap()[i * 128:(i + 1) * 128, :])
                nc.gpsimd.dma_start(wh[i][:],
                                    WhT.ap()[i * 128:(i + 1) * 128, :])
                nc.gpsimd.dma_start(
                    ctile[i][:], c_sl.ap()[i * 128:(i + 1) * 128, :])
            nc.gpsimd.dma_start(xa2[:], xT_aug.ap()[256:258, :])
            nc.gpsimd.dma_start(wi2[:], WiT.ap()[256:258, :])

            for bh in range(2):
                bsl = slice(bh * 128, (bh + 1) * 128)
                pre = psA.tile([128, GS], F32, tag="ps_scratch", name="pre")
                nc.tensor.matmul(pre[:], xa[0][:, bsl], wi[0][:],
                                 start=True, stop=False)
                nc.tensor.matmul(pre[:], xa[1][:, bsl], wi[1][:],
                                 start=False, stop=False)
                nc.tensor.matmul(pre[:], xa2[:, bsl], wi2[:],
                                 start=False, stop=False)
                nc.tensor.matmul(pre[:], ha[0][:, bsl], wh[0][:],
                                 start=False, stop=False)
                nc.tensor.matmul(pre[:], ha[1][:, bsl], wh[1][:],
                                 start=False, stop=True)
                gates = sbA.tile([128, GS], F32, tag="gates")
                # sigmoid(x) = exp(-ln(1 + exp(-x))): stays on the Ln/Exp ACT
                # table and avoids custom DVE ucode (reciprocal) entirely
                e1 = sbA.tile([128, 128], F32, tag="e1")
                nc.scalar.activation(e1[:], pre[:, 0:128], AF.Exp, scale=cm1[:])
                nc.vector.tensor_scalar_add(e1[:], e1[:], 1.0)
                l1 = sbA.tile([128, 128], F32, tag="l1")
                nc.scalar.activation(l1[:], e1[:], AF.Ln)
                nc.scalar.activation(gates[:, 0:128], l1[:], AF.Exp,
                                     scale=cm1[:])
                # tanh(x) = 2 * sigmoid(2x) - 1
                e2 = sbA.tile([128, HS], F32, tag="e2")
                nc.scalar.activation(e2[:], pre[:, 128:160], AF.Exp,
                                     scale=cm2[:])
                nc.vector.tensor_scalar_add(e2[:], e2[:], 1.0)
                l2 = sbA.tile([128, HS], F32, tag="l2")
                nc.scalar.activation(l2[:], e2[:], AF.Ln)
                e3 = sbA.tile([128, HS], F32, tag="e3")
                nc.scalar.activation(e3[:], l2[:], AF.Exp, scale=cm1[:])
                nc.vector.tensor_scalar(
                    gates[:, 128:160], e3[:], 2.0, -1.0,
                    op0=mybir.AluOpType.mult, op1=mybir.AluOpType.add)
                # c_part = f*c + i*c~
                fc = sbA.tile([128, HS], F32, tag="fc")
                nc.vector.tensor_mul(fc[:], gates[:, 0:HS], ctile[bh][:])
                ic = sbA.tile([128, HS], F32, tag="ic")
                nc.vector.tensor_mul(ic[:], gates[:, HS:2 * HS],
                                     gates[:, 128:160])
                cp = sbA.tile([128, HS], F32, tag="cp")
                nc.vector.tensor_add(cp[:], fc[:], ic[:])
                nc.gpsimd.dma_start(org.ap()[bsl, 0:HS],
                                    gates[:, 2 * HS:3 * HS])      # o
                nc.gpsimd.dma_start(org.ap()[bsl, HS:2 * HS],
                                    gates[:, 3 * HS:4 * HS])      # r
                nc.gpsimd.dma_start(org.ap()[bsl, 2 * HS:3 * HS], cp[:])

            # --- phase C: stream the kNN retrieval ---
            # Software-pipelined: block i+1's loads, casts and rsqrt-norm
            # chain (a ~9us serial latency: DMA -> cast -> sq -> PE ones-mm
            # -> copy -> reshape-DMA -> PE transpose -> ACT ln/exp) are
            # emitted BEFORE block i's tile loop, so the chain hides under
            # the previous block's matmul work and PE never idles long
            # enough for the HAM clock-gate to re-throttle.
            av = [avps.tile([128, H + 2], F32, tag=f"av{bh}", name=f"av{bh}")
                  for bh in range(2)]
            total_tiles = lpad // 128

            def emit_front(bi):
                """DMA + casts + rsqrt-norm chain for block bi."""
                bs = blocks[bi]
                off = bi * blk
                nt = bs // 128
                kt = kpool.tile([128, 2, bs], F32, tag="kt", name="kt")
                nc.sync.dma_start(
                    kt[:],
                    keysT.ap()[:, off:off + bs].rearrange(
                        "(c p) l -> p c l", p=128))
                kt16 = k16pool.tile([128, 2, bs], BF16, tag="kt16",
                                    name="kt16")
                nc.vector.tensor_copy(kt16[:], kt[:])
                vb = vbpool.tile([128, nt_max * (H + 2)], F32, tag="vb",
                                 name="vb")
                nc.sync.dma_start(vb[:, 0:nt * (H + 2)],
                                  vals2.ap()[bi, :, 0:nt * (H + 2)])
                vb16 = v16pool.tile([128, nt_max * (H + 2)], BF16, tag="vb16",
                                    name="vb16")
                nc.vector.tensor_copy(vb16[:, 0:nt * (H + 2)],
                                      vb[:, 0:nt * (H + 2)])
                sq = sqpool.tile([128, 2, bs], BF16, tag="sq", name="sq")
                nc.vector.tensor_mul(sq[:], kt16[:], kt16[:])
                # normsq[l] -> rsq[p, t] (= 1/||k_l||, l = 128*t + p):
                # chunk sums [1, 512] -> SBUF [1, bs] -> reshape-DMA
                # [nt, 128] -> tiny PE transpose -> [128, nt] psum -> rsqrt
                chunks = [(j0, min(512, bs - j0))
                          for j0 in range(0, bs, 512)]
                nqs = rqpool.tile([1, 2048], F32, tag="nqs", name="nqs")
                for j, (j0, cs) in enumerate(chunks):
                    nq = nqps.tile([1, 512], F32, tag="nq", name="nq")
                    nc.tensor.matmul(nq[:, 0:cs], ones32[:, 0:1],
                                     sq[:, 0, j0:j0 + cs],
                                     start=True, stop=False)
                    nc.tensor.matmul(nq[:, 0:cs], ones32[:, 0:1],
                                     sq[:, 1, j0:j0 + cs],
                                     start=False, stop=True)
                    if j % 2 == 0:
                        nc.vector.tensor_copy(nqs[:, j0:j0 + cs],
                                              nq[:, 0:cs])
                    else:
                        nc.scalar.copy(nqs[:, j0:j0 + cs], nq[:, 0:cs])
                t4 = rqpool.tile([16, 128], F32, tag="t4", name="t4")
                nc.gpsimd.dma_start(
                    t4[0:nt, :],
                    nqs[0:1, 0:bs].rearrange("o (r p) -> o r p", p=128))
                tpn = psA.tile([128, 16], F32, tag="ps_scratch", name="tpn")
                nc.tensor.transpose(tpn[:, 0:nt], t4[0:nt, :],
                                    ident[0:nt, 0:nt])
                rsq = rqpool.tile([128, nt_max], F32, tag="rsq", name="rsq")
                rln = rqpool.tile([128, nt_max], F32, tag="rln", name="rln")
                nc.scalar.activation(rln[:, 0:nt], tpn[:, 0:nt], AF.Ln)
                nc.scalar.activation(rsq[:, 0:nt], rln[:, 0:nt], AF.Exp,
                                     scale=cm05[:])
                return kt16, vb16, rsq

            state = emit_front(0)
            tile_idx = 0
            for bi, bs in enumerate(blocks):
                kt16, vb16, rsq = state
                if bi + 1 < len(blocks):
                    state = emit_front(bi + 1)
                off = bi * blk
                nt = bs // 128
                for t in range(nt):
                    l0 = off + t * 128
                    real = min(128, max(0, l_real - l0))
                    sm = smps.tile([128, B], F32, tag="sm")
                    nc.tensor.matmul(sm[:], kt16[:, 0, t * 128:(t + 1) * 128],
                                     qnT[0][:], start=True, stop=False)
                    nc.tensor.matmul(sm[:], kt16[:, 1, t * 128:(t + 1) * 128],
                                     qnT[1][:], start=False, stop=True)
                    ex = expool.tile([128, B], BF16, tag="ex")
                    nc.scalar.activation(ex[:], sm[:], AF.Exp,
                                         bias=cm1[:], scale=rsq[:, t:t + 1])
                    if real <= 0:
                        tile_idx += 1
                        continue
                    first = tile_idx == 0
                    last = tile_idx == total_tiles - 1
                    for bh in range(2):
                        nc.tensor.matmul(
                            av[bh][:],
                            ex[0:real, bh * 128:(bh + 1) * 128],
                            vb16[0:real, t * (H + 2):(t + 1) * (H + 2)],
                            start=first, stop=last)
                    tile_idx += 1

            for bh in range(2):
                avs = sbA.tile([128, H + 2], F32, tag="avs")
                nc.vector.tensor_copy(avs[:], av[bh][:])
                nc.sync.dma_start(nd.ap()[bh * 128:(bh + 1) * 128, :],
                                  avs[:])

    nc.compile()
    return nc


_NC_CACHE = {}


def _get_nc():
    if "nc" not in _NC_CACHE:
        _NC_CACHE["nc"] = _build()
    return _NC_CACHE["nc"]


def _shard_inputs(x_t, h, c, W_i2h, b_i2h, W_h2h, b_h2h, keys, vals):
    f = np.float32
    x_t = np.ascontiguousarray(np.asarray(x_t, f))
    h = np.asarray(h, f)
    c = np.asarray(c, f)
    W_i2h = np.asarray(W_i2h, f)
    b_i2h = np.asarray(b_i2h, f)
    W_h2h = np.asarray(W_h2h, f)
    b_h2h = np.asarray(b_h2h, f)
    keys = np.asarray(keys, f)
    vals = np.asarray(vals, f)

    xT_aug = np.ascontiguousarray(
        np.concatenate([x_t.T, np.ones((2, B), f)], axis=0))
    hT = np.ascontiguousarray(h.T)
    WiT_full = W_i2h.T  # [D, G]
    WhT_full = W_h2h.T  # [H, G]

    in_maps = []
    for k in range(NCORES):
        sl = slice(k * L_LOC, (k + 1) * L_LOC)
        keysT = np.zeros((D, LPAD), f)
        keysT[:, :L_LOC] = keys[sl].T
        keysT[0, L_LOC:] = 1.0  # dummy unit keys (excluded from the sums)
        vpad = np.zeros((LPAD, H + 2), f)
        vpad[:L_LOC, :H] = vals[sl]
        vpad[:L_LOC, H] = 1.0  # denominator column (excluded rows stay 0)
        nblk = (LPAD + BLK - 1) // BLK
        v2 = np.zeros((nblk, 128, NT_MAX * (H + 2)), f)
        for bi in range(nblk):
            bs = min(BLK, LPAD - bi * BLK)
            nt = bs // 128
            blkv = vpad[bi * BLK:bi * BLK + bs]          # [bs, 258]
            v2[bi, :, :nt * (H + 2)] = blkv.reshape(
                nt, 128, H + 2).transpose(1, 0, 2).reshape(128, nt * (H + 2))
        gcols = np.concatenate(
            [np.arange(j * H + k * HS, j * H + (k + 1) * HS)
             for j in range(5)])
        WiT = np.concatenate(
            [WiT_full[:, gcols], b_i2h[gcols][None, :],
             b_h2h[gcols][None, :]], axis=0)
        in_maps.append({
            "onesc": np.ones((128, 32), ml_dtypes.bfloat16),
            "keysT": np.ascontiguousarray(keysT),
            "vals2": v2,
            "x_t": x_t,
            "xT_aug": xT_aug,
            "hT": hT,
            "WiT": np.ascontiguousarray(WiT),
            "WhT": np.ascontiguousarray(WhT_full[:, gcols]),
            "c_sl": np.ascontiguousarray(c[:, k * HS:(k + 1) * HS]),
        })
    return in_maps


def kernel(x_t, h, c, W_i2h, b_i2h, W_h2h, b_h2h, keys, vals):
    nc = _get_nc()
    in_maps = _shard_inputs(x_t, h, c, W_i2h, b_i2h, W_h2h, b_h2h, keys, vals)
    res = bass_utils.run_bass_kernel_spmd(
        nc, in_maps, core_ids=list(range(NCORES)))

    num = np.zeros((B, H), np.float64)
    den = np.zeros((B,), np.float64)
    for k in range(NCORES):
        ndk = res.results[k]["nd"]
        num += ndk[:, :H]
        den += ndk[:, H]
    m = np.tanh(num / den[:, None]).astype(np.float32)

    h_t = np.empty((B, H), np.float32)
    c_t = np.empty((B, H), np.float32)
    for k in range(NCORES):
        orgk = res.results[k]["org"]
        o = orgk[:, 0:HS]
        r = orgk[:, HS:2 * HS]
        cp = orgk[:, 2 * HS:3 * HS]
        hs = slice(k * HS, (k + 1) * HS)
        ct = cp + r * m[:, hs]
        c_t[:, hs] = ct
        h_t[:, hs] = o * np.tanh(ct)
    return (h_t, c_t)
